# revision 1
# baseline (speedup 1.0000x reference)
"""Trainium2 Bass kernel for nn_EncodingLayer (2-layer GCN + encoder MLP).

Strategy (8 NeuronCores, SPMD):
  - Nodes sharded by destination range: core k owns nodes [k*12544, (k+1)*12544).
  - Host: append self-loops, compute deg/rsqrt/edge norms (numpy), partition
    edges per dst shard, order by (window-batch, src-quarter, window, src),
    pad each (window, quarter) group to a common (max-over-cores) chunk count
    so all 8 cores share one SPMD program layout.
  - Device per core, layer 1: gpsimd dma_gather of x_bf16[src] (int16 indices
    relative to 25088-row quarter blocks), per-chunk one-hot scatter matrices
    built on DVE as (iota==dstoff)*norm, scatter-add via TensorE matmul
    accumulation in PSUM (feature-major out), then h1 = tanh(agg @ W1 + b1),
    PE-transpose, store node-major.
  - AllGather h1 across the 8 cores (collective).
  - Layer 2: same gather/scatter from h1_full, adj/gdv/pr heads + encoder MLP.
    All sigmoids are computed as 0.5*tanh(0.5*z)+0.5 folded into the encoder
    weights (host-side) so the whole kernel uses one ACT table (Tanh+Copy).
  - Output written feature-major [128, 12544] per core; host transposes+concats.
"""

import numpy as np
import ml_dtypes

BF16 = ml_dtypes.bfloat16

N_NODES = 100000
N_EDGES = 1600000
D = 128
P = 128
N_CORES = 8
NW = 108                # windows (of 128 dst nodes) per core
SH = NW * P             # 13824 nodes per core (padded)
NPAD = N_CORES * SH     # 110592
NQ = 4                  # src quarters (int16 index range)
QS = NPAD // NQ         # 25088 rows per quarter
NBW = 8                 # windows per gather batch

_cache = {}

NREAL_Q = N_NODES // NQ     # 25000 real nodes per quarter


def _pos(n):
    """Map original node ids to padded positions: 25000 real nodes spread
    uniformly over each quarter's 26880 positions (pads interleaved so all
    cores/windows carry equal real-node counts)."""
    q, i = n // NREAL_Q, n % NREAL_Q
    return q * QS + (i * QS) // NREAL_Q



def _batches():
    b = 0
    while b < NW:
        e = min(b + NBW, NW)
        yield b, e
        b = e


def _plan(nchq):
    """Shared host/builder traversal plan.

    Returns per-batch dicts with:
      wlist, col0 (global chunk column of batch start),
      q_runs: per q: (col_off_global, n_chunks_q, [(w, nch)]),
      and per-window (start_key, stop_key) among (q, j) chunk keys.
    """
    plan = []
    col = 0
    for b, e in _batches():
        wlist = list(range(b, e))
        ent = {"wlist": wlist, "col0": col, "q_runs": []}
        for q in range(NQ):
            runs = [(w, int(nchq[w][q])) for w in wlist]
            nq = sum(r[1] for r in runs)
            ent["q_runs"].append((col, nq, runs))
            col += nq
        first = {}
        last = {}
        for q in range(NQ):
            for w in wlist:
                if nchq[w][q] > 0:
                    if w not in first:
                        first[w] = q
                    last[w] = q
        ent["first_q"] = first
        ent["last_q"] = last
        plan.append(ent)
    return plan, col  # col == C_total


def _host_prep(edge_index, edge_weight):
    src = np.concatenate([edge_index[0].astype(np.int64),
                          np.arange(N_NODES, dtype=np.int64)])
    dst = np.concatenate([edge_index[1].astype(np.int64),
                          np.arange(N_NODES, dtype=np.int64)])
    w = np.concatenate([edge_weight.astype(np.float32),
                        np.ones(N_NODES, np.float32)])

    src = _pos(src)
    dst = _pos(dst)
    deg = np.bincount(dst, weights=w, minlength=NPAD).astype(np.float32)
    with np.errstate(divide="ignore"):
        dinv = np.where(deg > 0, 1.0 / np.sqrt(np.maximum(deg, 1e-30)), 0.0)
    dinv = dinv.astype(np.float32)
    norm = (dinv[src] * w * dinv[dst]).astype(np.float32)

    nself_all = dinv * dinv                      # self-loop weight = 1
    ns_edge = np.arange(len(src)) < N_EDGES      # first N_EDGES are real edges
    src, dst, norm = src[ns_edge], dst[ns_edge], norm[ns_edge]

    core = dst // SH
    per_core = []
    counts = np.zeros((N_CORES, NW, NQ), dtype=np.int64)
    for k in range(N_CORES):
        m = core == k
        s_k, d_k, n_k = src[m], dst[m] - k * SH, norm[m]
        wnd = d_k >> 7
        qq = s_k // QS
        order = np.lexsort((s_k, qq, wnd))
        s_k, d_k, n_k, wnd, qq = (s_k[order], d_k[order], n_k[order],
                                  wnd[order], qq[order])
        idx2 = wnd * NQ + qq
        counts[k] = np.bincount(idx2, minlength=NW * NQ).reshape(NW, NQ)
        per_core.append((s_k, d_k, n_k))

    nchq = np.ceil(counts.max(axis=0) / P).astype(np.int64)   # [NW, NQ]
    plan, C_total = _plan(nchq)

    meta = []
    for k in range(N_CORES):
        s_k, d_k, n_k = per_core[k]
        cw = counts[k]
        # per (w, q) edge array offsets (edges are sorted by (w, q, src))
        offs = np.zeros((NW, NQ), dtype=np.int64)
        flat = cw.reshape(-1).cumsum()
        offs.reshape(-1)[1:] = flat[:-1]

        eidx16 = np.zeros(C_total * P, dtype=np.int16)
        edst = np.zeros((P, C_total), dtype=BF16)
        enrm = np.zeros((P, C_total), dtype=BF16)
        for ent in plan:
            for q in range(NQ):
                col_off, nq, runs = ent["q_runs"][q]
                c = col_off
                for wv, nch in runs:
                    if nch == 0:
                        continue
                    cnt = int(cw[wv, q])
                    o = int(offs[wv, q])
                    pad = nch * P
                    si = np.zeros(pad, np.int16)
                    so = np.zeros(pad, np.float32)
                    sn = np.zeros(pad, np.float32)
                    si[:cnt] = (s_k[o:o + cnt] - q * QS).astype(np.int16)
                    so[:cnt] = d_k[o:o + cnt] & 127
                    sn[:cnt] = n_k[o:o + cnt]
                    eidx16[c * P:(c + nch) * P] = si
                    edst[:, c:c + nch] = so.reshape(nch, P).T
                    enrm[:, c:c + nch] = sn.reshape(nch, P).T
                    c += nch
        # wrap into [128, C_total*8] int16 tile: position i -> [i%16, i//16],
        # replicated down the 8 groups of 16 partitions.
        wrapped = eidx16.reshape(-1, 16).T            # [16, C_total*8]
        idxw = np.ascontiguousarray(np.tile(wrapped, (8, 1)))  # [128, C*8]
        nself = np.ascontiguousarray(
            nself_all[k * SH:(k + 1) * SH].reshape(NW, P).T).astype(BF16)
        meta.append((idxw, edst, enrm, nself))
    return meta, nchq, plan, C_total


def _build(nchq, plan, C_total):
    import concourse.bacc as bacc
    import concourse.tile as tile
    import concourse.mybir as mybir
    from concourse import library_config

    dt = mybir.dt
    AF = mybir.ActivationFunctionType
    OP = mybir.AluOpType

    nc = bacc.Bacc("TRN2", target_bir_lowering=False, debug=False,
                   enable_asserts=False, num_devices=N_CORES)

    def din(name, shape, dty):
        return nc.dram_tensor(name, shape, dty, kind="ExternalInput").ap()

    x_bf = din("x_bf", [NPAD, D], dt.bfloat16)
    eidx_d = din("eidx", [P, C_total * 8], dt.int16)
    edst_d = din("edst", [P, C_total], dt.bfloat16)
    enrm_d = din("enrm", [P, C_total], dt.bfloat16)
    xown_d = din("xown", [SH, D], dt.bfloat16)
    nself_d = din("nself", [P, NW], dt.bfloat16)
    identb_d = din("identb", [P, P], dt.bfloat16)
    gdvT_d = din("gdvT", [73, SH], dt.bfloat16)
    prT_d = din("prT", [1, SH], dt.bfloat16)
    iota_d = din("iota", [P, P], dt.bfloat16)
    ident_d = din("ident", [P, P], dt.float32)
    W1_d = din("W1", [D, D], dt.bfloat16)
    b1_d = din("b1", [D, 1], dt.float32)
    W2_d = din("W2", [D, 64], dt.bfloat16)
    b2h_d = din("b2h", [64, 1], dt.float32)
    gdvW_d = din("gdvW", [73, 32], dt.bfloat16)
    gdvbh_d = din("gdvbh", [32, 1], dt.float32)
    prW_d = din("prW", [1, 32], dt.bfloat16)
    prbh_d = din("prbh", [32, 1], dt.float32)
    encW1_d = din("encW1", [D, D], dt.bfloat16)
    encb1_d = din("encb1", [D, 1], dt.float32)
    encW2_d = din("encW2", [D, D], dt.bfloat16)
    encb2_d = din("encb2", [D, 1], dt.float32)

    out_d = nc.dram_tensor("out", [D, SH], dt.float32, kind="ExternalOutput").ap()
    h1loc = nc.dram_tensor("h1loc", [SH, D], dt.bfloat16, kind="Internal").ap()
    h1full = nc.dram_tensor("h1full", [NPAD, D], dt.bfloat16, kind="Internal",
                            addr_space="Shared").ap()

    CBMAX = max(ent["q_runs"][NQ - 1][0] + ent["q_runs"][NQ - 1][1]
                - ent["col0"] for ent in plan)

    with tile.TileContext(nc) as tc:
        with (
            tc.tile_pool(name="const", bufs=1) as cpool,
            tc.tile_pool(name="msgs", bufs=2) as gpool,
            tc.tile_pool(name="oh", bufs=6) as ohpool,
            tc.tile_pool(name="work", bufs=3) as wpool,
            tc.tile_pool(name="psw", bufs=3, space="PSUM") as pwp,
            tc.tile_pool(name="psh", bufs=2, space="PSUM") as psh,
            tc.tile_pool(name="pst", bufs=2, space="PSUM") as pst,
        ):
            nc.gpsimd.load_library(library_config.mlp)

            def load_const(ap, shape, dty, tag):
                t = cpool.tile(shape, dtype=dty, tag=tag)
                nc.sync.dma_start(out=t[:], in_=ap)
                return t

            iota_sb = load_const(iota_d[:, :], [P, P], dt.bfloat16, "iota")
            nself_sb = load_const(nself_d[:, :], [P, NW], dt.bfloat16, "nself")
            identb_sb = load_const(identb_d[:, :], [P, P], dt.bfloat16, "identb")
            ident_sb = load_const(ident_d[:, :], [P, P], dt.float32, "ident")
            eidx_sb = load_const(eidx_d[:, :], [P, C_total * 8], dt.int16, "eidx")
            edst_sb = load_const(edst_d[:, :], [P, C_total], dt.bfloat16, "edst")
            enrm_sb = load_const(enrm_d[:, :], [P, C_total], dt.bfloat16, "enrm")
            gdvT_sb = load_const(gdvT_d[:, :], [73, SH], dt.bfloat16, "gdvT")
            prT_sb = load_const(prT_d[:, :], [1, SH], dt.bfloat16, "prT")
            W1_sb = load_const(W1_d[:, :], [D, D], dt.bfloat16, "W1")
            b1_sb = load_const(b1_d[:, :], [D, 1], dt.float32, "b1")
            W2_sb = load_const(W2_d[:, :], [D, 64], dt.bfloat16, "W2")
            b2h_sb = load_const(b2h_d[:, :], [64, 1], dt.float32, "b2h")
            gdvW_sb = load_const(gdvW_d[:, :], [73, 32], dt.bfloat16, "gdvW")
            gdvbh_sb = load_const(gdvbh_d[:, :], [32, 1], dt.float32, "gdvbh")
            prW_sb = load_const(prW_d[:, :], [1, 32], dt.bfloat16, "prW")
            prbh_sb = load_const(prbh_d[:, :], [32, 1], dt.float32, "prbh")
            encW1_sb = load_const(encW1_d[:, :], [D, D], dt.bfloat16, "encW1")
            encb1_sb = load_const(encb1_d[:, :], [D, 1], dt.float32, "encb1")
            encW2_sb = load_const(encW2_d[:, :], [D, D], dt.bfloat16, "encW2")
            encb2_sb = load_const(encb2_d[:, :], [D, 1], dt.float32, "encb2")

            def gcn_batch(ent, src_ap, src_own, tail_fn):
                """Gather batch (quarter-major), scatter per window, tail per window."""
                col0 = ent["col0"]
                wlist = ent["wlist"]
                msgs_t = gpool.tile([P, CBMAX, P], dtype=dt.bfloat16, tag="msgs")
                colmap = {}
                for q in range(NQ):
                    col_off, nq, runs = ent["q_runs"][q]
                    c = col_off
                    for wv, nch in runs:
                        colmap[(q, wv)] = c
                        c += nch
                    if nq == 0:
                        continue
                    lo = col_off - col0
                    ni = nq * P
                    nc.gpsimd.dma_gather(
                        msgs_t[:, lo:lo + nq, :],
                        src_ap[q * QS:(q + 1) * QS, :],
                        eidx_sb[:, col_off * 8:(col_off + nq) * 8],
                        ni, ni, P, single_packet=False)
                for wv in wlist:
                    psw = pwp.tile([P, P], dtype=dt.float32, tag="psw")
                    mw = gpool.tile([P, P], dtype=dt.bfloat16, tag="mself")
                    nc.sync.dma_start(out=mw[:],
                                      in_=src_own[wv * P:(wv + 1) * P, :])
                    ohd = ohpool.tile([P, P], dtype=dt.bfloat16, tag="oh")
                    nc.vector.tensor_tensor(
                        out=ohd[:], in0=identb_sb[:],
                        in1=nself_sb[:, wv:wv + 1].to_broadcast([P, P]),
                        op=OP.mult)
                    has_edges = any(int(nchq[wv][q]) > 0 for q in range(NQ))
                    nc.tensor.matmul(psw[:], lhsT=mw[:], rhs=ohd[:],
                                     start=True, stop=not has_edges)
                    for q in range(NQ):
                        nch = int(nchq[wv][q])
                        cs = colmap.get((q, wv))
                        for j in range(nch):
                            oh_t = ohpool.tile([P, P], dtype=dt.bfloat16, tag="oh")
                            oh0 = ohpool.tile([P, P], dtype=dt.bfloat16, tag="oh0")
                            nc.vector.tensor_tensor(
                                out=oh0[:], in0=iota_sb[:],
                                in1=edst_sb[:, cs + j:cs + j + 1].to_broadcast([P, P]),
                                op=OP.is_equal)
                            nc.vector.tensor_tensor(
                                out=oh_t[:], in0=oh0[:],
                                in1=enrm_sb[:, cs + j:cs + j + 1].to_broadcast([P, P]),
                                op=OP.mult)
                            nc.tensor.matmul(
                                psw[:],
                                lhsT=msgs_t[:, cs + j - col0, :], rhs=oh_t[:],
                                start=False,
                                stop=(ent["last_q"][wv] == q and j == nch - 1))
                    tail_fn(wv, psw[:])

            def l1_tail(wv, psw_ap):
                agg_sb = wpool.tile([P, P], dtype=dt.bfloat16, tag="agg")
                nc.scalar.copy(agg_sb[:], psw_ap)
                ph = psh.tile([P, P], dtype=dt.float32, tag="ph")
                nc.tensor.matmul(ph[:], lhsT=W1_sb[:], rhs=agg_sb[:],
                                 start=True, stop=True)
                h1_sb = wpool.tile([P, P], dtype=dt.float32, tag="h1")
                nc.scalar.activation(h1_sb[:], ph[:], AF.Tanh, bias=b1_sb[:, 0:1])
                pt = pst.tile([P, P], dtype=dt.float32, tag="pt")
                nc.tensor.transpose(pt[:], h1_sb[:], ident_sb[:])
                h1t_sb = wpool.tile([P, P], dtype=dt.bfloat16, tag="h1t")
                nc.vector.tensor_copy(out=h1t_sb[:], in_=pt[:])
                nc.sync.dma_start(out=h1loc[wv * P:(wv + 1) * P, :], in_=h1t_sb[:])

            def l2_tail(wv, psw_ap):
                agg_sb = wpool.tile([P, P], dtype=dt.bfloat16, tag="agg")
                nc.scalar.copy(agg_sb[:], psw_ap)
                ncol = slice(wv * P, (wv + 1) * P)
                enc_sb = wpool.tile([P, P], dtype=dt.bfloat16, tag="enc")
                pa = psh.tile([64, P], dtype=dt.float32, tag="ph")
                nc.tensor.matmul(pa[:], lhsT=W2_sb[:], rhs=agg_sb[:],
                                 start=True, stop=True)
                nc.scalar.activation(enc_sb[0:64, :], pa[:], AF.Tanh,
                                     bias=b2h_sb[:, 0:1], scale=0.5)
                pg = psh.tile([32, P], dtype=dt.float32, tag="ph")
                nc.tensor.matmul(pg[:], lhsT=gdvW_sb[:], rhs=gdvT_sb[:, ncol],
                                 start=True, stop=True)
                nc.scalar.activation(enc_sb[64:96, :], pg[:], AF.Tanh,
                                     bias=gdvbh_sb[:, 0:1], scale=0.5)
                pp = psh.tile([32, P], dtype=dt.float32, tag="ph")
                nc.tensor.matmul(pp[:], lhsT=prW_sb[:], rhs=prT_sb[:, ncol],
                                 start=True, stop=True)
                nc.scalar.activation(enc_sb[96:128, :], pp[:], AF.Tanh,
                                     bias=prbh_sb[:, 0:1], scale=0.5)
                pe1 = psh.tile([P, P], dtype=dt.float32, tag="ph")
                nc.tensor.matmul(pe1[:], lhsT=encW1_sb[:], rhs=enc_sb[:],
                                 start=True, stop=True)
                e1_sb = wpool.tile([P, P], dtype=dt.bfloat16, tag="e1")
                nc.scalar.activation(e1_sb[:], pe1[:], AF.Tanh,
                                     bias=encb1_sb[:, 0:1])
                po = psh.tile([P, P], dtype=dt.float32, tag="ph")
                nc.tensor.matmul(po[:], lhsT=encW2_sb[:], rhs=e1_sb[:],
                                 start=True, stop=True)
                out_sb = wpool.tile([P, P], dtype=dt.float32, tag="outw")
                nc.vector.tensor_scalar_add(out_sb[:], po[:], encb2_sb[:, 0:1])
                nc.sync.dma_start(out=out_d[:, ncol], in_=out_sb[:])

            for ent in plan:
                gcn_batch(ent, x_bf, xown_d, l1_tail)

            tc.strict_bb_all_engine_barrier()
            nc.gpsimd.collective_compute(
                "AllGather", OP.bypass,
                replica_groups=[list(range(N_CORES))],
                ins=[h1loc], outs=[h1full])
            tc.strict_bb_all_engine_barrier()

            for ent in plan:
                gcn_batch(ent, h1full, h1loc, l2_tail)
    nc.compile()
    return nc


def _prepare(inputs):
    feat = np.asarray(inputs["feat"], np.float32)
    gdv = np.asarray(inputs["gdv"], np.float32)
    pr = np.asarray(inputs["pr"], np.float32)
    edge_index = np.asarray(inputs["edge_index"])
    edge_weight = np.asarray(inputs["edge_weight"], np.float32)

    key = hash((edge_index.tobytes(), edge_weight.tobytes()))
    if key in _cache:
        meta, nc = _cache[key]
    else:
        meta, nchq, plan, C_total = _host_prep(edge_index, edge_weight)
        nc = _build(nchq, plan, C_total)
        _cache.clear()
        _cache[key] = (meta, nc)

    pos = _pos(np.arange(N_NODES))
    x_bf = np.zeros((NPAD, D), dtype=BF16)
    x_bf[pos] = feat.astype(BF16)
    gdv_p = np.zeros((NPAD, 73), dtype=BF16)
    gdv_p[pos] = gdv.astype(BF16)
    pr_p = np.zeros((NPAD, 1), dtype=BF16)
    pr_p[pos] = pr.astype(BF16)

    W1 = np.asarray(inputs["W1"], np.float32)
    b1 = np.asarray(inputs["b1"], np.float32)
    W2 = np.asarray(inputs["W2"], np.float32)
    b2 = np.asarray(inputs["b2"], np.float32)
    gdvW = np.asarray(inputs["gdv_W"], np.float32)
    gdvb = np.asarray(inputs["gdv_b"], np.float32)
    prW = np.asarray(inputs["pr_W"], np.float32)
    prb = np.asarray(inputs["pr_b"], np.float32)
    encW1 = np.asarray(inputs["enc_W1"], np.float32)
    encb1 = np.asarray(inputs["enc_b1"], np.float32)
    encW2 = np.asarray(inputs["enc_W2"], np.float32)
    encb2 = np.asarray(inputs["enc_b2"], np.float32)

    iota = np.broadcast_to(np.arange(P, dtype=np.float32), (P, P))
    common = {
        "x_bf": x_bf,
        "iota": np.ascontiguousarray(iota.astype(BF16)),
        "ident": np.eye(P, dtype=np.float32),
        "W1": W1.astype(BF16),
        "b1": b1.reshape(D, 1),
        "W2": W2.astype(BF16),
        "b2h": (0.5 * b2).reshape(64, 1),
        "gdvW": gdvW.astype(BF16),
        "gdvbh": (0.5 * gdvb).reshape(32, 1),
        "prW": prW.astype(BF16),
        "prbh": (0.5 * prb).reshape(32, 1),
        "encW1": (0.5 * encW1).astype(BF16),
        "encb1": (encb1 + 0.5 * encW1.sum(0)).reshape(D, 1),
        "encW2": encW2.astype(BF16),
        "encb2": encb2.reshape(D, 1),
    }
    in_maps = []
    for k in range(N_CORES):
        idxw, edst, enrm, nself = meta[k]
        sl = slice(k * SH, (k + 1) * SH)
        in_maps.append(dict(
            common,
            eidx=idxw, edst=edst, enrm=enrm, nself=nself,
            xown=np.ascontiguousarray(x_bf[sl]),
            identb=np.eye(P, dtype=np.float32).astype(BF16),
            gdvT=np.ascontiguousarray(gdv_p[sl].T),
            prT=np.ascontiguousarray(pr_p[sl].T),
        ))
    return nc, in_maps


def run(inputs, trace=False):
    import concourse.bass_utils as bass_utils
    nc, in_maps = _prepare(inputs)
    res = bass_utils.run_bass_kernel_spmd(
        nc, in_maps, core_ids=list(range(N_CORES)), trace=trace)
    out = np.zeros((NPAD, D), dtype=np.float32)
    for k in range(N_CORES):
        out[k * SH:(k + 1) * SH] = res.results[k]["out"].T
    return out[_pos(np.arange(N_NODES))], res


def kernel(**inputs):
    out, _ = run(inputs, trace=False)
    return out



# revision 7
# speedup vs baseline: 1.8631x; 1.8631x over previous
"""Trainium2 Bass kernel for nn_EncodingLayer (2-layer GCN + encoder MLP).

Strategy (8 NeuronCores, SPMD):
  - Nodes sharded by destination range: core k owns nodes [k*12544, (k+1)*12544).
  - Host: append self-loops, compute deg/rsqrt/edge norms (numpy), partition
    edges per dst shard, order by (window-batch, src-quarter, window, src),
    pad each (window, quarter) group to a common (max-over-cores) chunk count
    so all 8 cores share one SPMD program layout.
  - Device per core, layer 1: gpsimd dma_gather of x_bf16[src] (int16 indices
    relative to 25088-row quarter blocks), per-chunk one-hot scatter matrices
    built on DVE as (iota==dstoff)*norm, scatter-add via TensorE matmul
    accumulation in PSUM (feature-major out), then h1 = tanh(agg @ W1 + b1),
    PE-transpose, store node-major.
  - AllGather h1 across the 8 cores (collective).
  - Layer 2: same gather/scatter from h1_full, adj/gdv/pr heads + encoder MLP.
    All sigmoids are computed as 0.5*tanh(0.5*z)+0.5 folded into the encoder
    weights (host-side) so the whole kernel uses one ACT table (Tanh+Copy).
  - Output written feature-major [128, 12544] per core; host transposes+concats.
"""

import numpy as np
import ml_dtypes

BF16 = ml_dtypes.bfloat16

N_NODES = 100000
N_EDGES = 1600000
D = 128
P = 128
N_CORES = 8
NW = 108                # windows (of 128 dst nodes) per core
SH = NW * P             # 13824 nodes per core (padded)
NPAD = N_CORES * SH     # 110592
NQ = 4                  # src quarters (int16 index range)
QS = NPAD // NQ         # 25088 rows per quarter
NBW = 4                 # windows per gather batch

_cache = {}

NREAL_Q = N_NODES // NQ     # 25000 real nodes per quarter


def _pos(n):
    """Map original node ids to padded positions: 25000 real nodes spread
    uniformly over each quarter's 26880 positions (pads interleaved so all
    cores/windows carry equal real-node counts)."""
    q, i = n // NREAL_Q, n % NREAL_Q
    return q * QS + (i * QS) // NREAL_Q



def _batches():
    b = 0
    while b < NW:
        e = min(b + NBW, NW)
        yield b, e
        b = e


def _plan(nchq):
    """Shared host/builder traversal plan.

    Returns per-batch dicts with:
      wlist, col0 (global chunk column of batch start),
      q_runs: per q: (col_off_global, n_chunks_q, [(w, nch)]),
      and per-window (start_key, stop_key) among (q, j) chunk keys.
    """
    plan = []
    col = 0
    for b, e in _batches():
        wlist = list(range(b, e))
        ent = {"wlist": wlist, "col0": col, "q_runs": []}
        for q in range(NQ):
            runs = [(w, int(nchq[w][q])) for w in wlist]
            nq = sum(r[1] for r in runs)
            ent["q_runs"].append((col, nq, runs))
            col += nq
        first = {}
        last = {}
        for q in range(NQ):
            for w in wlist:
                if nchq[w][q] > 0:
                    if w not in first:
                        first[w] = q
                    last[w] = q
        ent["first_q"] = first
        ent["last_q"] = last
        plan.append(ent)
    return plan, col  # col == C_total


def _host_prep(edge_index, edge_weight):
    src = np.concatenate([edge_index[0].astype(np.int64),
                          np.arange(N_NODES, dtype=np.int64)])
    dst = np.concatenate([edge_index[1].astype(np.int64),
                          np.arange(N_NODES, dtype=np.int64)])
    w = np.concatenate([edge_weight.astype(np.float32),
                        np.ones(N_NODES, np.float32)])

    src = _pos(src)
    dst = _pos(dst)
    deg = np.bincount(dst, weights=w, minlength=NPAD).astype(np.float32)
    with np.errstate(divide="ignore"):
        dinv = np.where(deg > 0, 1.0 / np.sqrt(np.maximum(deg, 1e-30)), 0.0)
    dinv = dinv.astype(np.float32)
    norm = (dinv[src] * w * dinv[dst]).astype(np.float32)

    nself_all = dinv * dinv                      # self-loop weight = 1
    ns_edge = np.arange(len(src)) < N_EDGES      # first N_EDGES are real edges
    src, dst, norm = src[ns_edge], dst[ns_edge], norm[ns_edge]

    core = dst // SH
    per_core = []
    counts = np.zeros((N_CORES, NW, NQ), dtype=np.int64)
    for k in range(N_CORES):
        m = core == k
        s_k, d_k, n_k = src[m], dst[m] - k * SH, norm[m]
        wnd = d_k >> 7
        qq = s_k // QS
        order = np.lexsort((s_k, qq, wnd))
        s_k, d_k, n_k, wnd, qq = (s_k[order], d_k[order], n_k[order],
                                  wnd[order], qq[order])
        idx2 = wnd * NQ + qq
        counts[k] = np.bincount(idx2, minlength=NW * NQ).reshape(NW, NQ)
        per_core.append((s_k, d_k, n_k))

    nchq = np.ceil(counts.max(axis=0) / P).astype(np.int64)   # [NW, NQ]
    plan, C_total = _plan(nchq)

    meta = []
    for k in range(N_CORES):
        s_k, d_k, n_k = per_core[k]
        cw = counts[k]
        # per (w, q) edge array offsets (edges are sorted by (w, q, src))
        offs = np.zeros((NW, NQ), dtype=np.int64)
        flat = cw.reshape(-1).cumsum()
        offs.reshape(-1)[1:] = flat[:-1]

        eidx16 = np.zeros(C_total * P, dtype=np.int16)
        edst = np.zeros((P, C_total), dtype=BF16)
        enrm = np.zeros((P, C_total), dtype=BF16)
        for ent in plan:
            for q in range(NQ):
                col_off, nq, runs = ent["q_runs"][q]
                c = col_off
                for wv, nch in runs:
                    if nch == 0:
                        continue
                    cnt = int(cw[wv, q])
                    o = int(offs[wv, q])
                    pad = nch * P
                    si = np.zeros(pad, np.int16)
                    so = np.zeros(pad, np.float32)
                    sn = np.zeros(pad, np.float32)
                    si[:cnt] = (s_k[o:o + cnt] - q * QS).astype(np.int16)
                    so[:cnt] = d_k[o:o + cnt] & 127
                    sn[:cnt] = n_k[o:o + cnt]
                    eidx16[c * P:(c + nch) * P] = si
                    edst[:, c:c + nch] = so.reshape(nch, P).T
                    enrm[:, c:c + nch] = sn.reshape(nch, P).T
                    c += nch
        # wrap into [128, C_total*8] int16 tile: position i -> [i%16, i//16],
        # replicated down the 8 groups of 16 partitions.
        wrapped = eidx16.reshape(-1, 16).T            # [16, C_total*8]
        idxw = np.ascontiguousarray(np.tile(wrapped, (8, 1)))  # [128, C*8]
        nself = np.ascontiguousarray(
            nself_all[k * SH:(k + 1) * SH].reshape(NW, P).T).astype(BF16)
        meta.append((idxw, edst, enrm, nself))
    return meta, nchq, plan, C_total


def _build(nchq, plan, C_total):
    import concourse.bacc as bacc
    import concourse.tile as tile
    import concourse.mybir as mybir
    from concourse import library_config

    dt = mybir.dt
    AF = mybir.ActivationFunctionType
    OP = mybir.AluOpType

    nc = bacc.Bacc("TRN2", target_bir_lowering=False, debug=False,
                   enable_asserts=False, num_devices=N_CORES,
                   num_swdge_queues=4)

    def din(name, shape, dty):
        return nc.dram_tensor(name, shape, dty, kind="ExternalInput").ap()

    x_bf = din("x_bf", [NPAD, D], dt.bfloat16)
    eidx_d = din("eidx", [P, C_total * 8], dt.int16)
    edst_d = din("edst", [P, C_total], dt.bfloat16)
    enrm_d = din("enrm", [P, C_total], dt.bfloat16)
    xown_d = din("xown", [SH, D], dt.bfloat16)
    nself_d = din("nself", [P, NW], dt.bfloat16)
    identb_d = din("identb", [P, P], dt.bfloat16)
    gdvT_d = din("gdvT", [73, SH], dt.bfloat16)
    prT_d = din("prT", [1, SH], dt.bfloat16)
    iota_d = din("iota", [P, P], dt.bfloat16)
    ident_d = din("ident", [P, P], dt.float32)
    W1_d = din("W1", [D, D], dt.bfloat16)
    b1_d = din("b1", [D, 1], dt.float32)
    W2_d = din("W2", [D, 64], dt.bfloat16)
    b2h_d = din("b2h", [64, 1], dt.float32)
    gdvW_d = din("gdvW", [73, 32], dt.bfloat16)
    gdvbh_d = din("gdvbh", [32, 1], dt.float32)
    prW_d = din("prW", [1, 32], dt.bfloat16)
    prbh_d = din("prbh", [32, 1], dt.float32)
    encW1_d = din("encW1", [D, D], dt.bfloat16)
    encb1_d = din("encb1", [D, 1], dt.float32)
    encW2_d = din("encW2", [D, D], dt.bfloat16)
    encb2_d = din("encb2", [D, 1], dt.float32)

    out_d = nc.dram_tensor("out", [D, SH], dt.float32, kind="ExternalOutput").ap()
    h1loc = nc.dram_tensor("h1loc", [SH, D], dt.bfloat16, kind="Internal").ap()
    h1full = nc.dram_tensor("h1full", [NPAD, D], dt.bfloat16, kind="Internal",
                            addr_space="Shared").ap()

    CBMAX = max(ent["q_runs"][NQ - 1][0] + ent["q_runs"][NQ - 1][1]
                - ent["col0"] for ent in plan)

    with tile.TileContext(nc) as tc:
        with (
            tc.tile_pool(name="const", bufs=1) as cpool,
            tc.tile_pool(name="msgs", bufs=2) as gpool,
            tc.tile_pool(name="ohb", bufs=2) as ohbpool,
            tc.tile_pool(name="oh", bufs=4) as ohpool,
            tc.tile_pool(name="work", bufs=3) as wpool,
            tc.tile_pool(name="psw", bufs=3, space="PSUM") as pwp,
            tc.tile_pool(name="psh", bufs=2, space="PSUM") as psh,
            tc.tile_pool(name="pst", bufs=2, space="PSUM") as pst,
        ):
            nc.gpsimd.load_library(library_config.mlp)

            def load_const(ap, shape, dty, tag):
                t = cpool.tile(shape, dtype=dty, tag=tag)
                nc.sync.dma_start(out=t[:], in_=ap)
                return t

            iota_sb = load_const(iota_d[:, :], [P, P], dt.bfloat16, "iota")
            nself_sb = load_const(nself_d[:, :], [P, NW], dt.bfloat16, "nself")
            identb_sb = load_const(identb_d[:, :], [P, P], dt.bfloat16, "identb")
            ident_sb = load_const(ident_d[:, :], [P, P], dt.float32, "ident")
            eidx_sb = load_const(eidx_d[:, :], [P, C_total * 8], dt.int16, "eidx")
            edst_sb = load_const(edst_d[:, :], [P, C_total], dt.bfloat16, "edst")
            enrm_sb = load_const(enrm_d[:, :], [P, C_total], dt.bfloat16, "enrm")
            gdvT_sb = load_const(gdvT_d[:, :], [73, SH], dt.bfloat16, "gdvT")
            prT_sb = load_const(prT_d[:, :], [1, SH], dt.bfloat16, "prT")
            W1_sb = load_const(W1_d[:, :], [D, D], dt.bfloat16, "W1")
            b1_sb = load_const(b1_d[:, :], [D, 1], dt.float32, "b1")
            W2_sb = load_const(W2_d[:, :], [D, 64], dt.bfloat16, "W2")
            b2h_sb = load_const(b2h_d[:, :], [64, 1], dt.float32, "b2h")
            gdvW_sb = load_const(gdvW_d[:, :], [73, 32], dt.bfloat16, "gdvW")
            gdvbh_sb = load_const(gdvbh_d[:, :], [32, 1], dt.float32, "gdvbh")
            prW_sb = load_const(prW_d[:, :], [1, 32], dt.bfloat16, "prW")
            prbh_sb = load_const(prbh_d[:, :], [32, 1], dt.float32, "prbh")
            encW1_sb = load_const(encW1_d[:, :], [D, D], dt.bfloat16, "encW1")
            encb1_sb = load_const(encb1_d[:, :], [D, 1], dt.float32, "encb1")
            encW2_sb = load_const(encW2_d[:, :], [D, D], dt.bfloat16, "encW2")
            encb2_sb = load_const(encb2_d[:, :], [D, 1], dt.float32, "encb2")

            def gcn_batch(ent, src_ap, src_own, tail_fn):
                """Gather batch (quarter-major), scatter per window, tail per window."""
                col0 = ent["col0"]
                wlist = ent["wlist"]
                msgs_t = gpool.tile([P, CBMAX, P], dtype=dt.bfloat16, tag="msgs")
                ohb = ohbpool.tile([P, CBMAX, P], dtype=dt.bfloat16, tag="ohb")
                colmap = {}
                for q in range(NQ):
                    col_off, nq, runs = ent["q_runs"][q]
                    c = col_off
                    for wv, nch in runs:
                        colmap[(q, wv)] = c
                        c += nch
                    if nq == 0:
                        continue
                    lo = col_off - col0
                    ni = nq * P
                    nc.gpsimd.dma_gather(
                        msgs_t[:, lo:lo + nq, :],
                        src_ap[q * QS:(q + 1) * QS, :],
                        eidx_sb[:, col_off * 8:(col_off + nq) * 8],
                        ni, ni, P, single_packet=False, queue_num=q)
                    # fold edge norms into the gathered messages (also zeroes
                    # pad rows, whose norm is 0)
                    nc.vector.tensor_tensor(
                        out=msgs_t[:, lo:lo + nq, :],
                        in0=msgs_t[:, lo:lo + nq, :],
                        in1=enrm_sb[:, col_off:col_off + nq]
                            .unsqueeze(2).to_broadcast([P, nq, P]),
                        op=OP.mult)
                    # batched one-hot build: ohb[:, lo+j, :] = (iota == edst_j)
                    nc.vector.tensor_tensor(
                        out=ohb[:, lo:lo + nq, :],
                        in0=iota_sb[:].unsqueeze(1).to_broadcast([P, nq, P]),
                        in1=edst_sb[:, col_off:col_off + nq]
                            .unsqueeze(2).to_broadcast([P, nq, P]),
                        op=OP.is_equal)
                for wv in wlist:
                    psw = pwp.tile([P, P], dtype=dt.float32, tag="psw")
                    mw = gpool.tile([P, P], dtype=dt.bfloat16, tag="mself")
                    nc.sync.dma_start(out=mw[:],
                                      in_=src_own[wv * P:(wv + 1) * P, :])
                    ohd = ohpool.tile([P, P], dtype=dt.bfloat16, tag="oh")
                    nc.vector.tensor_tensor(
                        out=ohd[:], in0=identb_sb[:],
                        in1=nself_sb[:, wv:wv + 1].to_broadcast([P, P]),
                        op=OP.mult)
                    has_edges = any(int(nchq[wv][q]) > 0 for q in range(NQ))
                    nc.tensor.matmul(psw[:], lhsT=mw[:], rhs=ohd[:],
                                     start=True, stop=not has_edges)
                    for q in range(NQ):
                        nch = int(nchq[wv][q])
                        cs = colmap.get((q, wv))
                        for j in range(nch):
                            nc.tensor.matmul(
                                psw[:],
                                lhsT=msgs_t[:, cs + j - col0, :],
                                rhs=ohb[:, cs + j - col0, :],
                                start=False,
                                stop=(ent["last_q"][wv] == q and j == nch - 1))
                    tail_fn(wv, psw[:])

            def l1_tail(wv, psw_ap):
                agg_sb = wpool.tile([P, P], dtype=dt.bfloat16, tag="agg")
                nc.scalar.copy(agg_sb[:], psw_ap)
                ph = psh.tile([P, P], dtype=dt.float32, tag="ph")
                nc.tensor.matmul(ph[:], lhsT=W1_sb[:], rhs=agg_sb[:],
                                 start=True, stop=True)
                h1_sb = wpool.tile([P, P], dtype=dt.float32, tag="h1")
                nc.scalar.activation(h1_sb[:], ph[:], AF.Tanh, bias=b1_sb[:, 0:1])
                pt = pst.tile([P, P], dtype=dt.float32, tag="pt")
                nc.tensor.transpose(pt[:], h1_sb[:], ident_sb[:])
                h1t_sb = wpool.tile([P, P], dtype=dt.bfloat16, tag="h1t")
                nc.vector.tensor_copy(out=h1t_sb[:], in_=pt[:])
                nc.sync.dma_start(out=h1loc[wv * P:(wv + 1) * P, :], in_=h1t_sb[:])

            def l2_tail(wv, psw_ap):
                agg_sb = wpool.tile([P, P], dtype=dt.bfloat16, tag="agg")
                nc.scalar.copy(agg_sb[:], psw_ap)
                ncol = slice(wv * P, (wv + 1) * P)
                enc_sb = wpool.tile([P, P], dtype=dt.bfloat16, tag="enc")
                pa = psh.tile([64, P], dtype=dt.float32, tag="ph")
                nc.tensor.matmul(pa[:], lhsT=W2_sb[:], rhs=agg_sb[:],
                                 start=True, stop=True)
                nc.scalar.activation(enc_sb[0:64, :], pa[:], AF.Tanh,
                                     bias=b2h_sb[:, 0:1], scale=0.5)
                pg = psh.tile([32, P], dtype=dt.float32, tag="ph")
                nc.tensor.matmul(pg[:], lhsT=gdvW_sb[:], rhs=gdvT_sb[:, ncol],
                                 start=True, stop=True)
                nc.scalar.activation(enc_sb[64:96, :], pg[:], AF.Tanh,
                                     bias=gdvbh_sb[:, 0:1], scale=0.5)
                pp = psh.tile([32, P], dtype=dt.float32, tag="ph")
                nc.tensor.matmul(pp[:], lhsT=prW_sb[:], rhs=prT_sb[:, ncol],
                                 start=True, stop=True)
                nc.scalar.activation(enc_sb[96:128, :], pp[:], AF.Tanh,
                                     bias=prbh_sb[:, 0:1], scale=0.5)
                pe1 = psh.tile([P, P], dtype=dt.float32, tag="ph")
                nc.tensor.matmul(pe1[:], lhsT=encW1_sb[:], rhs=enc_sb[:],
                                 start=True, stop=True)
                e1_sb = wpool.tile([P, P], dtype=dt.bfloat16, tag="e1")
                nc.scalar.activation(e1_sb[:], pe1[:], AF.Tanh,
                                     bias=encb1_sb[:, 0:1])
                po = psh.tile([P, P], dtype=dt.float32, tag="ph")
                nc.tensor.matmul(po[:], lhsT=encW2_sb[:], rhs=e1_sb[:],
                                 start=True, stop=True)
                out_sb = wpool.tile([P, P], dtype=dt.float32, tag="outw")
                nc.vector.tensor_scalar_add(out_sb[:], po[:], encb2_sb[:, 0:1])
                nc.sync.dma_start(out=out_d[:, ncol], in_=out_sb[:])

            for ent in plan:
                gcn_batch(ent, x_bf, xown_d, l1_tail)

            tc.strict_bb_all_engine_barrier()
            nc.gpsimd.collective_compute(
                "AllGather", OP.bypass,
                replica_groups=[list(range(N_CORES))],
                ins=[h1loc], outs=[h1full])
            tc.strict_bb_all_engine_barrier()

            for ent in plan:
                gcn_batch(ent, h1full, h1loc, l2_tail)
    nc.compile()
    return nc


def _prepare(inputs):
    feat = np.asarray(inputs["feat"], np.float32)
    gdv = np.asarray(inputs["gdv"], np.float32)
    pr = np.asarray(inputs["pr"], np.float32)
    edge_index = np.asarray(inputs["edge_index"])
    edge_weight = np.asarray(inputs["edge_weight"], np.float32)

    key = hash((edge_index.tobytes(), edge_weight.tobytes()))
    if key in _cache:
        meta, nc = _cache[key]
    else:
        meta, nchq, plan, C_total = _host_prep(edge_index, edge_weight)
        nc = _build(nchq, plan, C_total)
        _cache.clear()
        _cache[key] = (meta, nc)

    pos = _pos(np.arange(N_NODES))
    x_bf = np.zeros((NPAD, D), dtype=BF16)
    x_bf[pos] = feat.astype(BF16)
    gdv_p = np.zeros((NPAD, 73), dtype=BF16)
    gdv_p[pos] = gdv.astype(BF16)
    pr_p = np.zeros((NPAD, 1), dtype=BF16)
    pr_p[pos] = pr.astype(BF16)

    W1 = np.asarray(inputs["W1"], np.float32)
    b1 = np.asarray(inputs["b1"], np.float32)
    W2 = np.asarray(inputs["W2"], np.float32)
    b2 = np.asarray(inputs["b2"], np.float32)
    gdvW = np.asarray(inputs["gdv_W"], np.float32)
    gdvb = np.asarray(inputs["gdv_b"], np.float32)
    prW = np.asarray(inputs["pr_W"], np.float32)
    prb = np.asarray(inputs["pr_b"], np.float32)
    encW1 = np.asarray(inputs["enc_W1"], np.float32)
    encb1 = np.asarray(inputs["enc_b1"], np.float32)
    encW2 = np.asarray(inputs["enc_W2"], np.float32)
    encb2 = np.asarray(inputs["enc_b2"], np.float32)

    iota = np.broadcast_to(np.arange(P, dtype=np.float32), (P, P))
    common = {
        "x_bf": x_bf,
        "iota": np.ascontiguousarray(iota.astype(BF16)),
        "ident": np.eye(P, dtype=np.float32),
        "W1": W1.astype(BF16),
        "b1": b1.reshape(D, 1),
        "W2": W2.astype(BF16),
        "b2h": (0.5 * b2).reshape(64, 1),
        "gdvW": gdvW.astype(BF16),
        "gdvbh": (0.5 * gdvb).reshape(32, 1),
        "prW": prW.astype(BF16),
        "prbh": (0.5 * prb).reshape(32, 1),
        "encW1": (0.5 * encW1).astype(BF16),
        "encb1": (encb1 + 0.5 * encW1.sum(0)).reshape(D, 1),
        "encW2": encW2.astype(BF16),
        "encb2": encb2.reshape(D, 1),
    }
    in_maps = []
    for k in range(N_CORES):
        idxw, edst, enrm, nself = meta[k]
        sl = slice(k * SH, (k + 1) * SH)
        in_maps.append(dict(
            common,
            eidx=idxw, edst=edst, enrm=enrm, nself=nself,
            xown=np.ascontiguousarray(x_bf[sl]),
            identb=np.eye(P, dtype=np.float32).astype(BF16),
            gdvT=np.ascontiguousarray(gdv_p[sl].T),
            prT=np.ascontiguousarray(pr_p[sl].T),
        ))
    return nc, in_maps


def run(inputs, trace=False):
    import concourse.bass_utils as bass_utils
    nc, in_maps = _prepare(inputs)
    res = bass_utils.run_bass_kernel_spmd(
        nc, in_maps, core_ids=list(range(N_CORES)), trace=trace)
    out = np.zeros((NPAD, D), dtype=np.float32)
    for k in range(N_CORES):
        out[k * SH:(k + 1) * SH] = res.results[k]["out"].T
    return out[_pos(np.arange(N_NODES))], res


def kernel(**inputs):
    out, _ = run(inputs, trace=False)
    return out

